# revision 21
# baseline (speedup 1.0000x reference)
"""KroneckerMessage GNN message passing on 8 TRN2 NeuronCores — v2.

Redesign vs v1 (2.755 ms): the v1 profile showed the gpsimd engine 75%
busy dispatching 1764 per-subtile indirect DMAs (~1.1 us fixed cost each),
DVE 59% busy on per-subtile small ops, and PE stuck at mid-pstate.

v2 strategy (hardcoded for nn_KroneckerMessage):
- Host: balanced dst windows. Nodes are permuted (snake by in-degree) so
  every 128-node window has <= K*128 incident edges with K=16 (vs 18), and
  windows are contiguous in the permuted node space. One-hot scatter
  matrices and all gather indices are precomputed on the host.
- Node phase: bf16; each core computes h = relu(LN(nf @ W_node)) for its
  6272 nodes, writes h_part [6272,20] bf16 + htab_local [6272,128] (256-B
  rows for dma_gather), AllGathers h_part, then builds htab [50178,128]
  (two int16-indexable halves, each with a leading zero row).
- Edge phase per window: 3 batched dma_gather calls (hs-low-half, hs-high
  -half, hd window-local) replace 2*K indirect DMAs; merge-add of the two
  hs halves; ONE big DVE tensor_tensor builds kron [128, K, 512-padded]
  bf16 for all K subtiles; ONE blocked dma_start_transpose (xbar) yields
  kron^T chunks [128, K*4, 128] (replaces 4*K PE transposes + PSUM->SBUF
  copies); per subtile 4 accumulating bf16 matmuls vs W chunks [128,129]
  (col 128 = row-means => per-edge mean for free); LN stats batched per
  3-subtile group (ACT Square into bf16 + grouped DVE reduce); per-window
  batched var/rsqrt; per-subtile fused ACT relu((z-mu)*rstd); scatter-add
  via host-precomputed one-hot matmuls accumulated in PSUM; one flush per
  window.
"""
import math
import os

import numpy as np
import ml_dtypes

import concourse.bacc as bacc
import concourse.bass as bass
import concourse.tile as tile
from concourse import mybir
from concourse.bass_utils import run_bass_kernel_spmd
from concourse.masks import make_identity

N_CORES = 8
P = 128
D_NODE = 20
KRON = 400
KPAD = 512
NCH = 4
LN_EPS = 1e-5

# module-level knobs (test.py pokes these)
TRACE = False
TRACE_DIR = None
USE_SIM = False

_BUILD_CACHE = {}


# --------------------------------------------------------------------------
# host-side prep
# --------------------------------------------------------------------------
def _prep(node_feat, W_node, b_node, g_node, beta_node, W_kron, b_kron,
          g_kron, beta_kron, src, dst):
    N, GF = node_feat.shape
    OUT = W_kron.shape[1]
    OUTX = OUT + 1
    E = src.shape[0]
    assert GF % P == 0 and OUT == P and W_kron.shape[0] == KRON

    nodes_pc = int(math.ceil(N / (N_CORES * P))) * P
    npad = nodes_pc * N_CORES
    wpc = nodes_pc // P
    nwin = wpc * N_CORES
    half_n = npad // 2

    src = np.asarray(src, np.int64)
    dst = np.asarray(dst, np.int64)

    # --- balanced window assignment (snake by in-degree) ---
    deg = np.bincount(dst, minlength=npad)
    order = np.argsort(-deg, kind="stable")
    win_of = np.empty(npad, np.int64)
    slot_of = np.empty(npad, np.int64)
    for r in range((npad + nwin - 1) // nwin):
        chunk = order[r * nwin:(r + 1) * nwin]
        ids = np.arange(len(chunk))
        if r % 2 == 1:
            ids = nwin - 1 - ids
        win_of[chunk] = ids
        slot_of[chunk] = r
    pos = win_of * P + slot_of  # node -> permuted position

    # edges sorted by (window, src-half): lo-src edges occupy the first B
    # subtiles of a window, hi-src edges the rest -> one gather descriptor
    # per edge slot and int16-safe indices (pos < half_n per table).
    ewin = win_of[dst]
    ps_all = pos[src]
    key = ewin * 2 + (ps_all >= half_n)
    eorder = np.argsort(key, kind="stable")
    sw = ewin[eorder]
    shalf = (ps_all[eorder] >= half_n).astype(np.int64)
    cnt2 = np.bincount(key, minlength=2 * nwin).reshape(nwin, 2)
    B = int(math.ceil(cnt2[:, 0].max() / P))
    KH = int(math.ceil(cnt2[:, 1].max() / P))
    K = max(1, B + KH)
    slots_pw = K * P

    starts2 = np.concatenate([[0], np.cumsum(cnt2.ravel())])
    rank = np.arange(E, dtype=np.int64) - starts2[sw * 2 + shalf]
    slot = sw * slots_pw + shalf * (B * P) + rank

    ps = ps_all[eorder]
    pd = pos[dst[eorder]]
    idx_lo = np.zeros(nwin * B * P, np.int16)
    idx_hi = np.zeros(nwin * KH * P, np.int16)
    dloc = np.full(nwin * slots_pw, 255, np.int32)
    s_in_w = slot % slots_pw
    lo = shalf == 0
    li = (slot[lo] // slots_pw) * (B * P) + s_in_w[lo]
    idx_lo[li] = ps[lo].astype(np.int16)
    hi = (slot[~lo] // slots_pw) * (KH * P) + (s_in_w[~lo] - B * P)
    idx_hi[hi] = (ps[~lo] - half_n).astype(np.int16)
    dloc[slot] = (pd % P).astype(np.int32)

    # one-hot scatter [nwin, P(edge), K, P(node)] and its transpose
    # [nwin, P(node), K, P(edge)] for the hd select matmuls
    dl = dloc.reshape(nwin, K, P)
    ohb = (dl[:, :, :, None] == np.arange(P)[None, None, None, :])
    oh = np.ascontiguousarray(
        ohb.transpose(0, 2, 1, 3)).astype(ml_dtypes.bfloat16)
    ohT = np.ascontiguousarray(
        ohb.transpose(0, 3, 1, 2)).astype(ml_dtypes.bfloat16)

    # wrapped int16 index layout: idx i -> [i % 16 (tiled to 128), i // 16]
    def wrap(a, n):
        a = a.reshape(nwin, n * 8, 16).transpose(0, 2, 1)  # [nwin, 16, n*8]
        return np.tile(a, (1, 8, 1))  # [nwin, 128, n*8]

    idx_in = np.concatenate([wrap(idx_lo, B), wrap(idx_hi, KH)],
                            axis=2)  # [nwin, 128, K*8]
    idx_in = np.ascontiguousarray(idx_in)

    nf_perm = np.zeros((npad, GF), np.float32)
    nf_perm[pos[:N]] = np.asarray(node_feat, np.float32)
    nf_perm = nf_perm.astype(ml_dtypes.bfloat16)
    # pre-transposed per-core layout [P, FCH, nodes_pc] so the device loads
    # nf^T directly instead of PE-transposing every tile
    nfT_all = np.ascontiguousarray(
        nf_perm.reshape(N_CORES, nodes_pc, GF // P, P).transpose(0, 3, 2, 1))

    FCH = GF // P
    wn = np.asarray(W_node, np.float32).reshape(FCH, P, D_NODE)
    wn = np.ascontiguousarray(wn.transpose(1, 0, 2)).astype(ml_dtypes.bfloat16)

    # wk [P, NCH, OUT] bf16: row p of chunk c = W_kron[c*128+p] (0 if >=400)
    wkf = np.zeros((KPAD, OUT), np.float32)
    wkf[:KRON, :OUT] = np.asarray(W_kron, np.float32)
    wk = np.ascontiguousarray(
        wkf.reshape(NCH, P, OUT).transpose(1, 0, 2)).astype(
            ml_dtypes.bfloat16)

    bk_col = np.asarray(b_kron, np.float32).reshape(OUT, 1)

    flags = dict(
        has_bn=bool(np.any(np.asarray(b_node) != 0)),
        has_gn=bool(np.any(np.asarray(g_node) != 1)),
        has_betan=bool(np.any(np.asarray(beta_node) != 0)),
        has_bk=bool(np.any(np.asarray(b_kron) != 0)),
        has_gk=bool(np.any(np.asarray(g_kron) != 1)),
        has_betak=bool(np.any(np.asarray(beta_kron) != 0)),
    )
    cfg = dict(N=N, GF=GF, OUT=OUT, OUTX=OUTX, E=E, nodes_pc=nodes_pc,
               npad=npad, wpc=wpc, K=K, B=B, half_n=half_n, **flags)

    in_maps = []
    for c in range(N_CORES):
        m = dict(
            nf=nfT_all[c],
            wn=wn,
            wk=wk,
            idx_in=idx_in[c * wpc:(c + 1) * wpc],
            oh_in=oh[c * wpc:(c + 1) * wpc],
            ohT_in=ohT[c * wpc:(c + 1) * wpc],
            b_node=np.asarray(b_node, np.float32),
            g_node=np.asarray(g_node, np.float32),
            beta_node=np.asarray(beta_node, np.float32),
            bk=bk_col,
            g_kron=np.asarray(g_kron, np.float32).astype(ml_dtypes.bfloat16),
            beta_kron=np.asarray(beta_kron,
                                 np.float32).astype(ml_dtypes.bfloat16),
        )
        in_maps.append(m)
    return cfg, in_maps, pos


# --------------------------------------------------------------------------
# device program
# --------------------------------------------------------------------------
def _build(cfg):
    GF, OUT, OUTX = cfg["GF"], cfg["OUT"], cfg["OUTX"]
    nodes_pc, npad, wpc, K, B, half_n = (cfg["nodes_pc"], cfg["npad"],
                                         cfg["wpc"], cfg["K"], cfg["B"],
                                         cfg["half_n"])
    KH = K - B
    FCH = GF // P
    f32, bf16, i16 = mybir.dt.float32, mybir.dt.bfloat16, mybir.dt.int16
    # 512-col PSUM groups of up to 4 subtiles each for the z^T matmuls
    NG = (K + 3) // 4
    assert NG + 2 <= 8, "PSUM bank budget"

    nc = bacc.Bacc(num_devices=N_CORES)
    nf = nc.dram_tensor("nf", [P, FCH, nodes_pc], bf16, kind="ExternalInput")
    wn = nc.dram_tensor("wn", [P, FCH, D_NODE], bf16, kind="ExternalInput")
    wk = nc.dram_tensor("wk", [P, NCH, OUT], bf16, kind="ExternalInput")
    idx_in = nc.dram_tensor("idx_in", [wpc, P, K * 8], i16,
                            kind="ExternalInput")
    oh_in = nc.dram_tensor("oh_in", [wpc, P, K, P], bf16,
                           kind="ExternalInput")
    ohT_in = nc.dram_tensor("ohT_in", [wpc, P, K, P], bf16,
                            kind="ExternalInput")
    b_node = nc.dram_tensor("b_node", [D_NODE], f32, kind="ExternalInput")
    g_node = nc.dram_tensor("g_node", [D_NODE], f32, kind="ExternalInput")
    beta_node = nc.dram_tensor("beta_node", [D_NODE], f32,
                               kind="ExternalInput")
    bk = nc.dram_tensor("bk", [OUT, 1], f32, kind="ExternalInput")
    g_kron = nc.dram_tensor("g_kron", [OUT], bf16, kind="ExternalInput")
    beta_kron = nc.dram_tensor("beta_kron", [OUT], bf16,
                               kind="ExternalInput")


    out_part = nc.dram_tensor("out_part", [nodes_pc, OUT], f32,
                              kind="ExternalOutput")
    h_part = nc.dram_tensor("h_part", [nodes_pc, D_NODE], bf16)
    h_full = nc.dram_tensor("h_full", [npad, D_NODE], bf16,
                            addr_space="Shared")
    htab = nc.dram_tensor("htab", [npad, P], bf16)

    ntiles = nodes_pc // P

    # ---------------- phase 1: h = relu(LN(nf @ W_node + b)) --------------
    with tile.TileContext(nc) as tc:
        with (
            tc.tile_pool(name="hconst", bufs=1) as hconst,
            tc.tile_pool(name="hsb", bufs=3) as hsb,
            tc.tile_pool(name="hps", bufs=2, space="PSUM") as hps,
            tc.tile_pool(name="hsmall", bufs=4) as hsmall,
        ):
            wn_sb = hconst.tile([P, FCH, D_NODE], bf16)
            nc.sync.dma_start(out=wn_sb[:], in_=wn[:])
            eps_t = hconst.tile([P, 1], f32)
            nc.vector.memset(eps_t[:], LN_EPS)
            if cfg["has_bn"]:
                bn_b = hconst.tile([P, D_NODE], f32)
                nc.sync.dma_start(
                    out=bn_b[:],
                    in_=bass.AP(tensor=b_node, offset=0,
                                ap=[[0, P], [1, D_NODE]]))
            if cfg["has_gn"]:
                gn_b = hconst.tile([P, D_NODE], f32)
                nc.sync.dma_start(
                    out=gn_b[:],
                    in_=bass.AP(tensor=g_node, offset=0,
                                ap=[[0, P], [1, D_NODE]]))
            if cfg["has_betan"]:
                betan_b = hconst.tile([P, D_NODE], f32)
                nc.sync.dma_start(
                    out=betan_b[:],
                    in_=bass.AP(tensor=beta_node, offset=0,
                                ap=[[0, P], [1, D_NODE]]))

            h_stage = hconst.tile([P, ntiles, D_NODE], bf16)

            for t in range(ntiles):
                nfT = hsb.tile([P, FCH, P], bf16, tag="nfT")
                nc.sync.dma_start(out=nfT[:], in_=nf[:, :, t * P:(t + 1) * P])
                z_ps = hps.tile([P, D_NODE], f32, tag="z_ps")
                for c in range(FCH):
                    nc.tensor.matmul(out=z_ps[:], lhsT=nfT[:, c, :],
                                     rhs=wn_sb[:, c, :], start=(c == 0),
                                     stop=(c == FCH - 1))
                if cfg["has_bn"]:
                    z_sb = hsb.tile([P, D_NODE], f32, tag="z_sb")
                    nc.vector.tensor_add(out=z_sb[:], in0=z_ps[:], in1=bn_b[:])
                    z_in = z_sb
                else:
                    z_in = z_ps
                stats = hsmall.tile([P, 6], f32, tag="stats")
                nc.vector.bn_stats(out=stats[:], in_=z_in[:])
                mv = hsmall.tile([P, 2], f32, tag="mv")
                nc.vector.bn_aggr(out=mv[:], in_=stats[:])
                sd = hsmall.tile([P, 1], f32, tag="sd")
                nc.scalar.activation(out=sd[:], in_=mv[:, 1:2],
                                     func=mybir.ActivationFunctionType.Sqrt,
                                     bias=eps_t[:], scale=1.0)
                rstd = hsmall.tile([P, 1], f32, tag="rstd")
                nc.vector.reciprocal(out=rstd[:], in_=sd[:])
                nmr = hsmall.tile([P, 1], f32, tag="nmr")
                nc.vector.tensor_scalar(out=nmr[:], in0=mv[:, 0:1],
                                        scalar1=rstd[:, 0:1], scalar2=-1.0,
                                        op0=mybir.AluOpType.mult,
                                        op1=mybir.AluOpType.mult)
                simple = not (cfg["has_gn"] or cfg["has_betan"])
                func = (mybir.ActivationFunctionType.Relu if simple
                        else mybir.ActivationFunctionType.Identity)
                nc.scalar.activation(out=h_stage[:, t, :], in_=z_in[:],
                                     func=func, bias=nmr[:],
                                     scale=rstd[:, 0:1])
                if not simple:
                    if cfg["has_gn"]:
                        nc.vector.tensor_mul(out=h_stage[:, t, :],
                                             in0=h_stage[:, t, :],
                                             in1=gn_b[:])
                    if cfg["has_betan"]:
                        nc.vector.tensor_add(out=h_stage[:, t, :],
                                             in0=h_stage[:, t, :],
                                             in1=betan_b[:])
                    nc.vector.tensor_scalar_max(out=h_stage[:, t, :],
                                                in0=h_stage[:, t, :],
                                                scalar1=0.0)
            nc.sync.dma_start(
                out=h_part.rearrange("(t p) d -> p t d", p=P),
                in_=h_stage[:])

    # ---------------- collective: AllGather h ----------------------------
    with (
        nc.Block() as block,
        nc.semaphore("cc_sem") as cc_sem,
    ):
        @block.gpsimd
        def _(gpsimd):
            gpsimd.collective_compute(
                "AllGather",
                mybir.AluOpType.bypass,
                replica_groups=[list(range(N_CORES))],
                ins=[h_part[:]],
                outs=[h_full[:]],
            ).then_inc(cc_sem)
            gpsimd.wait_ge(cc_sem, 1)

    # ---------------- phase 2: edges --------------------------------------
    # Main MLP matmuls run W-stationary, producing z^T [OUT, edges] with
    # long 512-col streams (one PSUM bank per 4-subtile group). z^T is then
    # cast to bf16 (bias folded into the ACT pass) and xbar-transposed back
    # to edge-major for batched LN stats/finals and the scatter matmuls.
    with tile.TileContext(nc) as tc:
        with (
            tc.tile_pool(name="econst", bufs=1) as econst,
            tc.tile_pool(name="eg", bufs=3) as eg,
            tc.tile_pool(name="ek", bufs=2) as ek,
            tc.tile_pool(name="esmall", bufs=2) as esmall,
            tc.tile_pool(name="ez", bufs=2) as ez,
            tc.tile_pool(name="ey", bufs=2) as ey,
            tc.tile_pool(name="ezp", bufs=1, space="PSUM") as ezp,
            tc.tile_pool(name="eap", bufs=1, space="PSUM") as eap,
            tc.tile_pool(name="ehd", bufs=1, space="PSUM") as ehd,
        ):
            # htab build: the collective Block exits with an all-engine
            # barrier (after gpsimd waits on cc_sem), so h_full is complete.
            # Expand 40-B h rows to 256-B gather rows on-chip so both DMA
            # sides move contiguous data (a direct strided DRAM->DRAM write
            # of 40-B rows measures ~170us). Pad columns stay uninitialized:
            # the gathers copy them but no compute ever reads them.
            JR = npad // P
            hsrc3 = h_full.rearrange("(p j) d -> p j d", p=P)
            htab3 = htab.rearrange("(p j) d -> p j d", p=P)
            NCHK = 16
            CW = (JR + NCHK - 1) // NCHK
            hx0 = econst.tile([P, CW, P], bf16, name="hx0")
            nc.vector.memset(hx0[:], 0.0)
            hx1 = econst.tile([P, CW, P], bf16, name="hx1")
            nc.vector.memset(hx1[:], 0.0)
            for ch in range(NCHK):
                j0 = ch * CW
                jn = min(CW, JR - j0)
                if jn <= 0:
                    break
                hin = ez.tile([P, CW, D_NODE], bf16, tag="hin")
                nc.sync.dma_start(out=hin[:, 0:jn, :],
                                  in_=hsrc3[:, j0:j0 + jn, :])
                hx = hx0 if ch % 2 == 0 else hx1
                nc.vector.tensor_copy(out=hx[:, 0:jn, 0:D_NODE],
                                      in_=hin[:, 0:jn, :])
                nc.sync.dma_start(out=htab3[:, j0:j0 + jn, :],
                                  in_=hx[:, 0:jn, :])

            wk_sb = econst.tile([P, NCH, OUT], bf16)
            nc.sync.dma_start(out=wk_sb[:], in_=wk[:])
            eps_t2 = econst.tile([P, 1], f32)
            nc.vector.memset(eps_t2[:], LN_EPS)
            if cfg["has_bk"]:
                bk_col = econst.tile([P, 1], f32)
                nc.sync.dma_start(out=bk_col[:], in_=bk[:, :])
            if cfg["has_gk"]:
                gk_b = econst.tile([P, OUT], bf16)
                nc.sync.dma_start(
                    out=gk_b[:], in_=bass.AP(tensor=g_kron, offset=0,
                                             ap=[[0, P], [1, OUT]]))
            if cfg["has_betak"]:
                betak_b = econst.tile([P, OUT], bf16)
                nc.sync.dma_start(
                    out=betak_b[:],
                    in_=bass.AP(tensor=beta_kron, offset=0,
                                ap=[[0, P], [1, OUT]]))

            # subtile ranges per PSUM group
            grps = [(4 * g, min(4, K - 4 * g)) for g in range(NG)]

            for w in range(wpc):
                idx3 = eg.tile([P, K * 8], i16, tag="idx3")
                nc.scalar.dma_start(out=idx3[:], in_=idx_in[w])
                oh_sb = eg.tile([P, K, P], bf16, tag="oh")
                nc.scalar.dma_start(out=oh_sb[:], in_=oh_in[w])
                ohT_sb = eg.tile([P, K, P], bf16, tag="ohT")
                nc.scalar.dma_start(out=ohT_sb[:], in_=ohT_in[w])
                hwin = eg.tile([P, D_NODE], bf16, tag="hwin")
                nc.scalar.dma_start(out=hwin[:],
                                    in_=h_part[w * P:(w + 1) * P, :])

                # hs: one descriptor per edge slot; lo/hi halves occupy
                # disjoint subtile ranges of the same tile
                hs = eg.tile([P, K, P], bf16, tag="hs")
                nc.gpsimd.dma_gather(
                    out_ap=hs[:, 0:B, :], in_ap=htab[0:half_n, :],
                    idxs_ap=idx3[:, 0:B * 8], num_idxs=B * P,
                    num_idxs_reg=B * P, elem_size=P, single_packet=False)
                nc.gpsimd.dma_gather(
                    out_ap=hs[:, B:K, :], in_ap=htab[half_n:, :],
                    idxs_ap=idx3[:, B * 8:K * 8], num_idxs=KH * P,
                    num_idxs_reg=KH * P, elem_size=P, single_packet=False)

                # hd: one-hot select from the window's own h rows (PE)
                hd_ps = ehd.tile([P, K, D_NODE], f32, tag="hd")
                for s_i in range(K):
                    nc.tensor.matmul(out=hd_ps[:, s_i, :],
                                     lhsT=ohT_sb[:, s_i, :], rhs=hwin[:],
                                     start=True, stop=True)
                hd_sb = eg.tile([P, K, D_NODE], bf16, tag="hd_sb")
                nc.vector.tensor_copy(out=hd_sb[:], in_=hd_ps[:])

                kron_all = ek.tile([P, K, KPAD], bf16, tag="kron")
                if w < 2:
                    nc.vector.memset(kron_all[:, :, KRON:KPAD], 0.0)
                kv = kron_all[:, :, 0:KRON].rearrange(
                    "p s (a b) -> p s a b", a=D_NODE)
                nc.vector.tensor_tensor(
                    out=kv,
                    in0=hs[:, :, 0:D_NODE, None].to_broadcast(
                        [P, K, D_NODE, D_NODE]),
                    in1=hd_sb[:, :, None, :].to_broadcast(
                        [P, K, D_NODE, D_NODE]),
                    op=mybir.AluOpType.mult)

                psT = ek.tile([P, K * NCH, P], bf16, tag="psT")
                nc.sync.dma_start_transpose(
                    out=psT[:], in_=kron_all[:].rearrange("p s k -> p (s k)"))
                psT4 = psT[:].rearrange("p (s c) e -> p s c e", c=NCH)

                # z^T = W_kron^T @ kron^T, W stationary, 512-col streams.
                # One PSUM bank per group; accumulate over the 4 contraction
                # chunks with the chunk loop OUTER so weights stay loaded.
                zg = [ezp.tile([P, 512], f32, tag=f"zg{g}", name=f"zg{g}")
                      for g in range(NG)]
                for g, (s0, ns) in enumerate(grps):
                    for c in range(NCH):
                        nc.tensor.matmul(
                            out=zg[g][:, 0:ns * P],
                            lhsT=wk_sb[:, c, :],
                            rhs=psT4[:, s0:s0 + ns, c, :],
                            start=(c == 0), stop=(c == NCH - 1))

                # cast z^T -> bf16 SBUF; b_kron is per-partition here, so it
                # folds into the ACT bias for free
                zt_sb = ez.tile([P, K * P], bf16, tag="zt_sb")
                for g, (s0, ns) in enumerate(grps):
                    nc.scalar.activation(
                        out=zt_sb[:, s0 * P:(s0 + ns) * P],
                        in_=zg[g][:, 0:ns * P],
                        func=mybir.ActivationFunctionType.Identity,
                        bias=(bk_col[:] if cfg["has_bk"] else 0.0),
                        scale=1.0)

                # transpose back to edge-major [edge, subtile, OUT].
                # Same queue as the kron transpose: the xbar is one physical
                # unit, so keep the two transposes serialized.
                z_ep = ez.tile([P, K, P], bf16, tag="z_ep")
                nc.sync.dma_start_transpose(out=z_ep[:], in_=zt_sb[:])

                # batched LN stats over the OUT axis
                ssum = esmall.tile([P, K], f32, tag="ssum")
                nc.vector.tensor_reduce(out=ssum[:], in_=z_ep[:],
                                        axis=mybir.AxisListType.X,
                                        op=mybir.AluOpType.add)
                nmu = esmall.tile([P, K], f32, tag="nmu")
                nc.vector.tensor_scalar(out=nmu[:], in0=ssum[:],
                                        scalar1=float(-1.0 / OUT),
                                        scalar2=None,
                                        op0=mybir.AluOpType.mult)
                sq = ez.tile([P, K, P], bf16, tag="sq")
                nc.scalar.activation(
                    out=sq[:].rearrange("p s e -> p (s e)"),
                    in_=z_ep[:].rearrange("p s e -> p (s e)"),
                    func=mybir.ActivationFunctionType.Square,
                    scale=float(1.0 / math.sqrt(OUT)))
                ss = esmall.tile([P, K], f32, tag="ss")
                nc.vector.tensor_reduce(out=ss[:], in_=sq[:],
                                        axis=mybir.AxisListType.X,
                                        op=mybir.AluOpType.add)
                var = esmall.tile([P, K], f32, tag="var")
                nc.vector.tensor_tensor(out=var[:], in0=nmu[:], in1=nmu[:],
                                        op=mybir.AluOpType.mult)
                nc.vector.tensor_tensor(out=var[:], in0=ss[:], in1=var[:],
                                        op=mybir.AluOpType.subtract)
                sd = esmall.tile([P, K], f32, tag="sd2")
                nc.scalar.activation(
                    out=sd[:], in_=var[:],
                    func=mybir.ActivationFunctionType.Sqrt,
                    bias=eps_t2[:], scale=1.0)
                rstd = esmall.tile([P, K], f32, tag="rstd2")
                nc.vector.reciprocal(out=rstd[:], in_=sd[:])

                # batched finals: y = relu((z + nmu) * rstd [* g + beta])
                t1 = ez.tile([P, K, P], bf16, tag="t1")
                nc.vector.tensor_tensor(
                    out=t1[:], in0=z_ep[:],
                    in1=nmu[:, :, None].to_broadcast([P, K, P]),
                    op=mybir.AluOpType.add)
                y_all = ez.tile([P, K, P], bf16, tag="y_all")
                nc.vector.tensor_tensor(
                    out=y_all[:], in0=t1[:],
                    in1=rstd[:, :, None].to_broadcast([P, K, P]),
                    op=mybir.AluOpType.mult)
                if cfg["has_gk"]:
                    nc.vector.tensor_tensor(
                        out=y_all[:], in0=y_all[:],
                        in1=gk_b[:, None, :].to_broadcast([P, K, P]),
                        op=mybir.AluOpType.mult)
                if cfg["has_betak"]:
                    nc.vector.tensor_tensor(
                        out=y_all[:], in0=y_all[:],
                        in1=betak_b[:, None, :].to_broadcast([P, K, P]),
                        op=mybir.AluOpType.add)
                nc.scalar.activation(
                    out=y_all[:].rearrange("p s e -> p (s e)"),
                    in_=y_all[:].rearrange("p s e -> p (s e)"),
                    func=mybir.ActivationFunctionType.Relu)

                # scatter-add to the window's nodes via one-hot matmuls
                acc_ps = eap.tile([P, OUT], f32, tag="acc")
                for s in range(K):
                    nc.tensor.matmul(out=acc_ps[:],
                                     lhsT=oh_sb[:, s, :],
                                     rhs=y_all[:, s, :], start=(s == 0),
                                     stop=(s == K - 1))

                out_sb = ey.tile([P, OUT], f32, tag="out_sb")
                nc.vector.tensor_copy(out=out_sb[:], in_=acc_ps[:])
                nc.scalar.dma_start(out=out_part[w * P:(w + 1) * P, :],
                                    in_=out_sb[:])

    nc.compile()
    return nc


# --------------------------------------------------------------------------
# entry point
# --------------------------------------------------------------------------
def _install_trace_hook():
    import sys, types, ctypes, contextlib
    if "antenv.axon_hooks" in sys.modules:
        return
    lib = ctypes.CDLL("/opt/axon/libaxon_pjrt.so")
    lib.axon_start_nrt_profile.argtypes = [ctypes.POINTER(ctypes.c_int64),
                                           ctypes.c_size_t]
    lib.axon_start_nrt_profile.restype = ctypes.c_int64
    lib.axon_stop_nrt_profile.argtypes = [ctypes.c_char_p]
    lib.axon_stop_nrt_profile.restype = ctypes.c_int64

    @contextlib.contextmanager
    def _hook(output_dir, device_ids):
        import jax
        jax.devices()
        if device_ids:
            ids = (ctypes.c_int64 * len(device_ids))(*device_ids)
            rc = lib.axon_start_nrt_profile(ids, len(device_ids))
        else:
            rc = lib.axon_start_nrt_profile(None, 0)
        if rc != 0:
            raise RuntimeError(f"axon_start_nrt_profile rc={rc}")
        try:
            yield
        finally:
            n = lib.axon_stop_nrt_profile(str(output_dir).encode())
            print(f"profile: {n} file(s) -> {output_dir}")

    mod = types.ModuleType("antenv.axon_hooks")
    mod.get_axon_ntff_profile_hook = lambda: _hook
    sys.modules["antenv.axon_hooks"] = mod
    from concourse import bass_utils
    bass_utils.upload_artifacts = lambda tmpdir: "local://skipped"


def kernel(**inputs):
    cfg, in_maps, pos = _prep(**inputs)
    key = (cfg["N"], cfg["GF"], cfg["OUT"], cfg["K"], cfg["has_bn"],
           cfg["has_gn"], cfg["has_betan"], cfg["has_bk"], cfg["has_gk"],
           cfg["has_betak"])
    if key not in _BUILD_CACHE:
        _BUILD_CACHE[key] = _build(cfg)
    nc = _BUILD_CACHE[key]

    if USE_SIM:
        from concourse import bass_interp
        sim = bass_interp.MultiCoreSim(nc, N_CORES)
        for c in range(N_CORES):
            for name, arr in in_maps[c].items():
                sim.cores[c].tensor(name)[:] = arr
        sim.simulate()
        parts = [np.array(sim.cores[c].tensor("out_part"))
                 for c in range(N_CORES)]
        exec_ns = None
    else:
        kw = {}
        if TRACE:
            _install_trace_hook()
            kw = dict(trace=True, tmpdir=TRACE_DIR)
        res = run_bass_kernel_spmd(nc, in_maps, list(range(N_CORES)), **kw)
        parts = [res.results[c]["out_part"] for c in range(N_CORES)]
        exec_ns = res.exec_time_ns
        kernel.last_exec_ns = exec_ns

    full = np.concatenate(parts, axis=0)
    out = full[pos[:cfg["N"]]]
    return out.astype(np.float32)


kernel.last_exec_ns = None



# revision 30
# speedup vs baseline: 1.0452x; 1.0452x over previous
"""KroneckerMessage GNN message passing on 8 TRN2 NeuronCores — v2.

Redesign vs v1 (2.755 ms): the v1 profile showed the gpsimd engine 75%
busy dispatching 1764 per-subtile indirect DMAs (~1.1 us fixed cost each),
DVE 59% busy on per-subtile small ops, and PE stuck at mid-pstate.

v2 strategy (hardcoded for nn_KroneckerMessage):
- Host: balanced dst windows. Nodes are permuted (snake by in-degree) so
  every 128-node window has <= K*128 incident edges with K=16 (vs 18), and
  windows are contiguous in the permuted node space. One-hot scatter
  matrices and all gather indices are precomputed on the host.
- Node phase: bf16; each core computes h = relu(LN(nf @ W_node)) for its
  6272 nodes, writes h_part [6272,20] bf16 + htab_local [6272,128] (256-B
  rows for dma_gather), AllGathers h_part, then builds htab [50178,128]
  (two int16-indexable halves, each with a leading zero row).
- Edge phase per window: 3 batched dma_gather calls (hs-low-half, hs-high
  -half, hd window-local) replace 2*K indirect DMAs; merge-add of the two
  hs halves; ONE big DVE tensor_tensor builds kron [128, K, 512-padded]
  bf16 for all K subtiles; ONE blocked dma_start_transpose (xbar) yields
  kron^T chunks [128, K*4, 128] (replaces 4*K PE transposes + PSUM->SBUF
  copies); per subtile 4 accumulating bf16 matmuls vs W chunks [128,129]
  (col 128 = row-means => per-edge mean for free); LN stats batched per
  3-subtile group (ACT Square into bf16 + grouped DVE reduce); per-window
  batched var/rsqrt; per-subtile fused ACT relu((z-mu)*rstd); scatter-add
  via host-precomputed one-hot matmuls accumulated in PSUM; one flush per
  window.
"""
import math
import os

import numpy as np
import ml_dtypes

import concourse.bacc as bacc
import concourse.bass as bass
import concourse.tile as tile
from concourse import mybir
from concourse.bass_utils import run_bass_kernel_spmd
from concourse.masks import make_identity

N_CORES = 8
P = 128
D_NODE = 20
KRON = 400
KPAD = 512
NCH = 4
LN_EPS = 1e-5

# module-level knobs (test.py pokes these)
TRACE = False
TRACE_DIR = None
USE_SIM = False

_BUILD_CACHE = {}


# --------------------------------------------------------------------------
# host-side prep
# --------------------------------------------------------------------------
def _prep(node_feat, W_node, b_node, g_node, beta_node, W_kron, b_kron,
          g_kron, beta_kron, src, dst):
    N, GF = node_feat.shape
    OUT = W_kron.shape[1]
    OUTX = OUT + 1
    E = src.shape[0]
    assert GF % P == 0 and OUT == P and W_kron.shape[0] == KRON

    nodes_pc = int(math.ceil(N / (N_CORES * P))) * P
    npad = nodes_pc * N_CORES
    wpc = nodes_pc // P
    nwin = wpc * N_CORES
    half_n = npad // 2

    src = np.asarray(src, np.int64)
    dst = np.asarray(dst, np.int64)

    # --- balanced window assignment (snake by in-degree) ---
    deg = np.bincount(dst, minlength=npad)
    order = np.argsort(-deg, kind="stable")
    win_of = np.empty(npad, np.int64)
    slot_of = np.empty(npad, np.int64)
    for r in range((npad + nwin - 1) // nwin):
        chunk = order[r * nwin:(r + 1) * nwin]
        ids = np.arange(len(chunk))
        if r % 2 == 1:
            ids = nwin - 1 - ids
        win_of[chunk] = ids
        slot_of[chunk] = r
    pos = win_of * P + slot_of  # node -> permuted position

    # edges sorted by (window, src-half): lo-src edges occupy the first B
    # subtiles of a window, hi-src edges the rest -> one gather descriptor
    # per edge slot and int16-safe indices (pos < half_n per table).
    ewin = win_of[dst]
    ps_all = pos[src]
    key = ewin * 2 + (ps_all >= half_n)
    eorder = np.argsort(key, kind="stable")
    sw = ewin[eorder]
    shalf = (ps_all[eorder] >= half_n).astype(np.int64)
    cnt2 = np.bincount(key, minlength=2 * nwin).reshape(nwin, 2)
    B = int(math.ceil(cnt2[:, 0].max() / P))
    KH = int(math.ceil(cnt2[:, 1].max() / P))
    K = max(1, B + KH)
    slots_pw = K * P

    starts2 = np.concatenate([[0], np.cumsum(cnt2.ravel())])
    rank = np.arange(E, dtype=np.int64) - starts2[sw * 2 + shalf]
    slot = sw * slots_pw + shalf * (B * P) + rank

    ps = ps_all[eorder]
    pd = pos[dst[eorder]]
    idx_lo = np.zeros(nwin * B * P, np.int16)
    idx_hi = np.zeros(nwin * KH * P, np.int16)
    dloc = np.full(nwin * slots_pw, 255, np.int32)
    s_in_w = slot % slots_pw
    lo = shalf == 0
    li = (slot[lo] // slots_pw) * (B * P) + s_in_w[lo]
    idx_lo[li] = ps[lo].astype(np.int16)
    hi = (slot[~lo] // slots_pw) * (KH * P) + (s_in_w[~lo] - B * P)
    idx_hi[hi] = (ps[~lo] - half_n).astype(np.int16)
    dloc[slot] = (pd % P).astype(np.int32)

    # one-hot scatter [nwin, P(edge), K, P(node)] and its transpose
    # [nwin, P(node), K, P(edge)] for the hd select matmuls
    dl = dloc.reshape(nwin, K, P)
    ohb = (dl[:, :, :, None] == np.arange(P)[None, None, None, :])
    oh = np.ascontiguousarray(
        ohb.transpose(0, 2, 1, 3)).astype(ml_dtypes.bfloat16)
    ohT = np.ascontiguousarray(
        ohb.transpose(0, 3, 1, 2)).astype(ml_dtypes.bfloat16)

    # wrapped int16 index layout: idx i -> [i % 16 (tiled to 128), i // 16]
    def wrap(a, n):
        a = a.reshape(nwin, n * 8, 16).transpose(0, 2, 1)  # [nwin, 16, n*8]
        return np.tile(a, (1, 8, 1))  # [nwin, 128, n*8]

    idx_in = np.concatenate([wrap(idx_lo, B), wrap(idx_hi, KH)],
                            axis=2)  # [nwin, 128, K*8]
    idx_in = np.ascontiguousarray(idx_in)

    nf_perm = np.zeros((npad, GF), np.float32)
    nf_perm[pos[:N]] = np.asarray(node_feat, np.float32)
    nf_perm = nf_perm.astype(ml_dtypes.bfloat16)
    # pre-transposed per-core layout [P, FCH, nodes_pc] so the device loads
    # nf^T directly instead of PE-transposing every tile
    nfT_all = np.ascontiguousarray(
        nf_perm.reshape(N_CORES, nodes_pc, GF // P, P).transpose(0, 3, 2, 1))

    FCH = GF // P
    wn = np.asarray(W_node, np.float32).reshape(FCH, P, D_NODE)
    wn = np.ascontiguousarray(wn.transpose(1, 0, 2)).astype(ml_dtypes.bfloat16)

    # wk [P, NCH, OUT] bf16: row p of chunk c = W_kron[c*128+p] (0 if >=400)
    wkf = np.zeros((KPAD, OUT), np.float32)
    wkf[:KRON, :OUT] = np.asarray(W_kron, np.float32)
    wk = np.ascontiguousarray(
        wkf.reshape(NCH, P, OUT).transpose(1, 0, 2)).astype(
            ml_dtypes.bfloat16)

    bk_col = np.asarray(b_kron, np.float32).reshape(OUT, 1)

    flags = dict(
        has_bn=bool(np.any(np.asarray(b_node) != 0)),
        has_gn=bool(np.any(np.asarray(g_node) != 1)),
        has_betan=bool(np.any(np.asarray(beta_node) != 0)),
        has_bk=bool(np.any(np.asarray(b_kron) != 0)),
        has_gk=bool(np.any(np.asarray(g_kron) != 1)),
        has_betak=bool(np.any(np.asarray(beta_kron) != 0)),
    )
    cfg = dict(N=N, GF=GF, OUT=OUT, OUTX=OUTX, E=E, nodes_pc=nodes_pc,
               npad=npad, wpc=wpc, K=K, B=B, half_n=half_n, **flags)

    in_maps = []
    for c in range(N_CORES):
        m = dict(
            nf=nfT_all[c],
            wn=wn,
            wk=wk,
            idx_in=idx_in[c * wpc:(c + 1) * wpc],
            oh_in=oh[c * wpc:(c + 1) * wpc],
            ohT_in=ohT[c * wpc:(c + 1) * wpc],
            b_node=np.asarray(b_node, np.float32),
            g_node=np.asarray(g_node, np.float32),
            beta_node=np.asarray(beta_node, np.float32),
            bk=bk_col,
            g_kron=np.asarray(g_kron, np.float32).astype(ml_dtypes.bfloat16),
            beta_kron=np.asarray(beta_kron,
                                 np.float32).astype(ml_dtypes.bfloat16),
        )
        in_maps.append(m)
    return cfg, in_maps, pos


# --------------------------------------------------------------------------
# device program
# --------------------------------------------------------------------------
def _build(cfg):
    GF, OUT, OUTX = cfg["GF"], cfg["OUT"], cfg["OUTX"]
    nodes_pc, npad, wpc, K, B, half_n = (cfg["nodes_pc"], cfg["npad"],
                                         cfg["wpc"], cfg["K"], cfg["B"],
                                         cfg["half_n"])
    KH = K - B
    FCH = GF // P
    f32, bf16, i16 = mybir.dt.float32, mybir.dt.bfloat16, mybir.dt.int16


    nc = bacc.Bacc(num_devices=N_CORES)
    nf = nc.dram_tensor("nf", [P, FCH, nodes_pc], bf16, kind="ExternalInput")
    wn = nc.dram_tensor("wn", [P, FCH, D_NODE], bf16, kind="ExternalInput")
    wk = nc.dram_tensor("wk", [P, NCH, OUT], bf16, kind="ExternalInput")
    idx_in = nc.dram_tensor("idx_in", [wpc, P, K * 8], i16,
                            kind="ExternalInput")
    oh_in = nc.dram_tensor("oh_in", [wpc, P, K, P], bf16,
                           kind="ExternalInput")
    ohT_in = nc.dram_tensor("ohT_in", [wpc, P, K, P], bf16,
                            kind="ExternalInput")
    b_node = nc.dram_tensor("b_node", [D_NODE], f32, kind="ExternalInput")
    g_node = nc.dram_tensor("g_node", [D_NODE], f32, kind="ExternalInput")
    beta_node = nc.dram_tensor("beta_node", [D_NODE], f32,
                               kind="ExternalInput")
    bk = nc.dram_tensor("bk", [OUT, 1], f32, kind="ExternalInput")
    g_kron = nc.dram_tensor("g_kron", [OUT], bf16, kind="ExternalInput")
    beta_kron = nc.dram_tensor("beta_kron", [OUT], bf16,
                               kind="ExternalInput")


    out_part = nc.dram_tensor("out_part", [nodes_pc, OUT], f32,
                              kind="ExternalOutput")
    h_part = nc.dram_tensor("h_part", [nodes_pc, D_NODE], bf16)
    h_full = nc.dram_tensor("h_full", [npad, D_NODE], bf16,
                            addr_space="Shared")
    htab = nc.dram_tensor("htab", [npad, P], bf16)

    ntiles = nodes_pc // P

    # ---------------- phase 1: h = relu(LN(nf @ W_node + b)) --------------
    with tile.TileContext(nc) as tc:
        with (
            tc.tile_pool(name="hconst", bufs=1) as hconst,
            tc.tile_pool(name="hsb", bufs=3) as hsb,
            tc.tile_pool(name="hps", bufs=2, space="PSUM") as hps,
            tc.tile_pool(name="hsmall", bufs=4) as hsmall,
        ):
            wn_sb = hconst.tile([P, FCH, D_NODE], bf16)
            nc.sync.dma_start(out=wn_sb[:], in_=wn[:])
            eps_t = hconst.tile([P, 1], f32)
            nc.vector.memset(eps_t[:], LN_EPS)
            if cfg["has_bn"]:
                bn_b = hconst.tile([P, D_NODE], f32)
                nc.sync.dma_start(
                    out=bn_b[:],
                    in_=bass.AP(tensor=b_node, offset=0,
                                ap=[[0, P], [1, D_NODE]]))
            if cfg["has_gn"]:
                gn_b = hconst.tile([P, D_NODE], f32)
                nc.sync.dma_start(
                    out=gn_b[:],
                    in_=bass.AP(tensor=g_node, offset=0,
                                ap=[[0, P], [1, D_NODE]]))
            if cfg["has_betan"]:
                betan_b = hconst.tile([P, D_NODE], f32)
                nc.sync.dma_start(
                    out=betan_b[:],
                    in_=bass.AP(tensor=beta_node, offset=0,
                                ap=[[0, P], [1, D_NODE]]))

            h_stage = hconst.tile([P, ntiles, D_NODE], bf16)

            for t in range(ntiles):
                nfT = hsb.tile([P, FCH, P], bf16, tag="nfT")
                nc.sync.dma_start(out=nfT[:], in_=nf[:, :, t * P:(t + 1) * P])
                z_ps = hps.tile([P, D_NODE], f32, tag="z_ps")
                for c in range(FCH):
                    nc.tensor.matmul(out=z_ps[:], lhsT=nfT[:, c, :],
                                     rhs=wn_sb[:, c, :], start=(c == 0),
                                     stop=(c == FCH - 1))
                if cfg["has_bn"]:
                    z_sb = hsb.tile([P, D_NODE], f32, tag="z_sb")
                    nc.vector.tensor_add(out=z_sb[:], in0=z_ps[:], in1=bn_b[:])
                    z_in = z_sb
                else:
                    z_in = z_ps
                stats = hsmall.tile([P, 6], f32, tag="stats")
                nc.vector.bn_stats(out=stats[:], in_=z_in[:])
                mv = hsmall.tile([P, 2], f32, tag="mv")
                nc.vector.bn_aggr(out=mv[:], in_=stats[:])
                sd = hsmall.tile([P, 1], f32, tag="sd")
                nc.scalar.activation(out=sd[:], in_=mv[:, 1:2],
                                     func=mybir.ActivationFunctionType.Sqrt,
                                     bias=eps_t[:], scale=1.0)
                rstd = hsmall.tile([P, 1], f32, tag="rstd")
                nc.vector.reciprocal(out=rstd[:], in_=sd[:])
                nmr = hsmall.tile([P, 1], f32, tag="nmr")
                nc.vector.tensor_scalar(out=nmr[:], in0=mv[:, 0:1],
                                        scalar1=rstd[:, 0:1], scalar2=-1.0,
                                        op0=mybir.AluOpType.mult,
                                        op1=mybir.AluOpType.mult)
                simple = not (cfg["has_gn"] or cfg["has_betan"])
                func = (mybir.ActivationFunctionType.Relu if simple
                        else mybir.ActivationFunctionType.Identity)
                nc.scalar.activation(out=h_stage[:, t, :], in_=z_in[:],
                                     func=func, bias=nmr[:],
                                     scale=rstd[:, 0:1])
                if not simple:
                    if cfg["has_gn"]:
                        nc.vector.tensor_mul(out=h_stage[:, t, :],
                                             in0=h_stage[:, t, :],
                                             in1=gn_b[:])
                    if cfg["has_betan"]:
                        nc.vector.tensor_add(out=h_stage[:, t, :],
                                             in0=h_stage[:, t, :],
                                             in1=betan_b[:])
                    nc.vector.tensor_scalar_max(out=h_stage[:, t, :],
                                                in0=h_stage[:, t, :],
                                                scalar1=0.0)
            nc.sync.dma_start(
                out=h_part.rearrange("(t p) d -> p t d", p=P),
                in_=h_stage[:])

    # ---------------- collective: AllGather h ----------------------------
    with (
        nc.Block() as block,
        nc.semaphore("cc_sem") as cc_sem,
    ):
        @block.gpsimd
        def _(gpsimd):
            gpsimd.collective_compute(
                "AllGather",
                mybir.AluOpType.bypass,
                replica_groups=[list(range(N_CORES))],
                ins=[h_part[:]],
                outs=[h_full[:]],
            ).then_inc(cc_sem)
            gpsimd.wait_ge(cc_sem, 1)

    # ---------------- phase 2: edges --------------------------------------
    # Main MLP matmuls run W-stationary, producing z^T [OUT, edges] with
    # long 512-col streams (one PSUM bank per 4-subtile group). z^T is then
    # cast to bf16 (bias folded into the ACT pass) and xbar-transposed back
    # to edge-major for batched LN stats/finals and the scatter matmuls.
    with tile.TileContext(nc) as tc:
        with (
            tc.tile_pool(name="econst", bufs=1) as econst,
            tc.tile_pool(name="eg", bufs=3) as eg,
            tc.tile_pool(name="ek", bufs=2) as ek,
            tc.tile_pool(name="esmall", bufs=2) as esmall,
            tc.tile_pool(name="ez", bufs=3) as ez,
            tc.tile_pool(name="ey", bufs=2) as ey,
            tc.tile_pool(name="ezp", bufs=1, space="PSUM") as ezp,
            tc.tile_pool(name="eap", bufs=1, space="PSUM") as eap,
            tc.tile_pool(name="ehd", bufs=1, space="PSUM") as ehd,
        ):
            # htab build: the collective Block exits with an all-engine
            # barrier (after gpsimd waits on cc_sem), so h_full is complete.
            # Expand 40-B h rows to 256-B gather rows on-chip so both DMA
            # sides move contiguous data (a direct strided DRAM->DRAM write
            # of 40-B rows measures ~170us). Pad columns stay uninitialized:
            # the gathers copy them but no compute ever reads them.
            JR = npad // P
            hsrc3 = h_full.rearrange("(p j) d -> p j d", p=P)
            htab3 = htab.rearrange("(p j) d -> p j d", p=P)
            NCHK = 16
            CW = (JR + NCHK - 1) // NCHK
            hx0 = econst.tile([P, CW, P], bf16, name="hx0")
            nc.vector.memset(hx0[:], 0.0)
            hx1 = econst.tile([P, CW, P], bf16, name="hx1")
            nc.vector.memset(hx1[:], 0.0)
            for ch in range(NCHK):
                j0 = ch * CW
                jn = min(CW, JR - j0)
                if jn <= 0:
                    break
                hin = ez.tile([P, CW, D_NODE], bf16, tag="hin")
                nc.sync.dma_start(out=hin[:, 0:jn, :],
                                  in_=hsrc3[:, j0:j0 + jn, :])
                hx = hx0 if ch % 2 == 0 else hx1
                nc.vector.tensor_copy(out=hx[:, 0:jn, 0:D_NODE],
                                      in_=hin[:, 0:jn, :])
                nc.sync.dma_start(out=htab3[:, j0:j0 + jn, :],
                                  in_=hx[:, 0:jn, :])

            wk_sb = econst.tile([P, NCH, OUT], bf16)
            nc.sync.dma_start(out=wk_sb[:], in_=wk[:])
            eps_t2 = econst.tile([P, 1], f32)
            nc.vector.memset(eps_t2[:], LN_EPS)
            if cfg["has_bk"]:
                bk_col = econst.tile([P, 1], f32)
                nc.sync.dma_start(out=bk_col[:], in_=bk[:, :])
            if cfg["has_gk"]:
                gk_b = econst.tile([P, OUT], bf16)
                nc.sync.dma_start(
                    out=gk_b[:], in_=bass.AP(tensor=g_kron, offset=0,
                                             ap=[[0, P], [1, OUT]]))
            if cfg["has_betak"]:
                betak_b = econst.tile([P, OUT], bf16)
                nc.sync.dma_start(
                    out=betak_b[:],
                    in_=bass.AP(tensor=beta_kron, offset=0,
                                ap=[[0, P], [1, OUT]]))

            # Halves aligned to the lo/hi gather split so each half flows
            # through kron-build -> xbar -> mains -> cast -> xbar -> LN ->
            # scatter as soon as its own gather lands. Groups (one PSUM bank
            # each) are balanced within each half.
            halves = [h for h in ((0, B), (B, KH)) if h[1] > 0]
            grps = []           # (s0, ns, half_index)
            for hi_, (h0, nh) in enumerate(halves):
                ngh = (nh + 3) // 4
                base, rem = divmod(nh, ngh)
                s = h0
                for gi in range(ngh):
                    ns = base + (1 if gi < rem else 0)
                    grps.append((s, ns, hi_))
                    s += ns
            NG2 = len(grps)
            assert NG2 + 2 <= 8, "PSUM bank budget"

            for w in range(wpc):
                idx3 = eg.tile([P, K * 8], i16, tag="idx3")
                nc.sync.dma_start(out=idx3[:], in_=idx_in[w])
                oh_sb = eg.tile([P, K, P], bf16, tag="oh")
                nc.scalar.dma_start(out=oh_sb[:], in_=oh_in[w])
                ohT_sb = eg.tile([P, K, P], bf16, tag="ohT")
                nc.scalar.dma_start(out=ohT_sb[:], in_=ohT_in[w])
                hwin = eg.tile([P, D_NODE], bf16, tag="hwin")
                nc.sync.dma_start(out=hwin[:],
                                  in_=h_part[w * P:(w + 1) * P, :])

                # hs: one descriptor per edge slot; lo/hi halves occupy
                # disjoint subtile ranges of the same tile
                hs = eg.tile([P, K, P], bf16, tag="hs")
                nc.gpsimd.dma_gather(
                    out_ap=hs[:, 0:B, :], in_ap=htab[0:half_n, :],
                    idxs_ap=idx3[:, 0:B * 8], num_idxs=B * P,
                    num_idxs_reg=B * P, elem_size=P, single_packet=False)
                nc.gpsimd.dma_gather(
                    out_ap=hs[:, B:K, :], in_ap=htab[half_n:, :],
                    idxs_ap=idx3[:, B * 8:K * 8], num_idxs=KH * P,
                    num_idxs_reg=KH * P, elem_size=P, single_packet=False)

                # hd: one-hot select from the window's own h rows (PE)
                hd_ps = ehd.tile([P, K, D_NODE], f32, tag="hd")
                for s_i in range(K):
                    nc.tensor.matmul(out=hd_ps[:, s_i, :],
                                     lhsT=ohT_sb[:, s_i, :], rhs=hwin[:],
                                     start=True, stop=True)
                hd_sb = eg.tile([P, K, D_NODE], bf16, tag="hd_sb")
                nc.vector.tensor_copy(out=hd_sb[:], in_=hd_ps[:])

                kron_all = ek.tile([P, K, KPAD], bf16, tag="kron")
                if w < 2:
                    nc.vector.memset(kron_all[:, :, KRON:KPAD], 0.0)
                psT = ek.tile([P, K * NCH, P], bf16, tag="psT")
                psT4 = psT[:].rearrange("p (s c) e -> p s c e", c=NCH)

                zg = [ezp.tile([P, 512], f32, tag=f"zg{g}", name=f"zg{g}")
                      for g in range(NG2)]
                acc_ps = eap.tile([P, OUT], f32, tag="acc")
                for hi_, (h0, nh) in enumerate(halves):
                    sfx = f"h{hi_}"
                    kv = kron_all[:, h0:h0 + nh, 0:KRON].rearrange(
                        "p s (a b) -> p s a b", a=D_NODE)
                    nc.vector.tensor_tensor(
                        out=kv,
                        in0=hs[:, h0:h0 + nh, 0:D_NODE, None].to_broadcast(
                            [P, nh, D_NODE, D_NODE]),
                        in1=hd_sb[:, h0:h0 + nh, None, :].to_broadcast(
                            [P, nh, D_NODE, D_NODE]),
                        op=mybir.AluOpType.mult)

                    nc.sync.dma_start_transpose(
                        out=psT[:, h0 * NCH:(h0 + nh) * NCH, :],
                        in_=kron_all[:, h0:h0 + nh, :].rearrange(
                            "p s k -> p (s k)"))

                    # z^T = W_kron^T @ kron^T, W stationary, long streams,
                    # one PSUM bank per group
                    for g, (s0, ns, gh) in enumerate(grps):
                        if gh != hi_:
                            continue
                        for c in range(NCH):
                            nc.tensor.matmul(
                                out=zg[g][:, 0:ns * P],
                                lhsT=wk_sb[:, c, :],
                                rhs=psT4[:, s0:s0 + ns, c, :],
                                start=(c == 0), stop=(c == NCH - 1))

                    # cast z^T -> bf16 SBUF; b_kron is per-partition here,
                    # so it folds into the ACT bias for free
                    zt_sb = ez.tile([P, nh * P], bf16, tag=f"zt_{sfx}",
                                    name=f"zt_{sfx}")
                    for g, (s0, ns, gh) in enumerate(grps):
                        if gh != hi_:
                            continue
                        o0 = (s0 - h0) * P
                        nc.scalar.activation(
                            out=zt_sb[:, o0:o0 + ns * P],
                            in_=zg[g][:, 0:ns * P],
                            func=mybir.ActivationFunctionType.Identity,
                            bias=(bk_col[:] if cfg["has_bk"] else 0.0),
                            scale=1.0)

                    # transpose back to edge-major [edge, subtile, OUT].
                    # Same queue as the kron transpose: the xbar is one
                    # physical unit, keep all transposes serialized.
                    z_ep = ez.tile([P, nh, P], bf16, tag=f"zep_{sfx}",
                                   name=f"zep_{sfx}")
                    nc.sync.dma_start_transpose(out=z_ep[:], in_=zt_sb[:])

                    # batched LN stats over the OUT axis
                    ssum = esmall.tile([P, nh], f32, tag=f"ssum_{sfx}",
                                       name=f"ssum_{sfx}")
                    nc.vector.tensor_reduce(out=ssum[:], in_=z_ep[:],
                                            axis=mybir.AxisListType.X,
                                            op=mybir.AluOpType.add)
                    nmu = esmall.tile([P, nh], f32, tag=f"nmu_{sfx}",
                                      name=f"nmu_{sfx}")
                    nc.vector.tensor_scalar(out=nmu[:], in0=ssum[:],
                                            scalar1=float(-1.0 / OUT),
                                            scalar2=None,
                                            op0=mybir.AluOpType.mult)
                    sq = ez.tile([P, nh, P], bf16, tag=f"sq_{sfx}",
                                 name=f"sq_{sfx}")
                    nc.scalar.activation(
                        out=sq[:].rearrange("p s e -> p (s e)"),
                        in_=z_ep[:].rearrange("p s e -> p (s e)"),
                        func=mybir.ActivationFunctionType.Square,
                        scale=float(1.0 / math.sqrt(OUT)))
                    ss = esmall.tile([P, nh], f32, tag=f"ss_{sfx}",
                                     name=f"ss_{sfx}")
                    nc.vector.tensor_reduce(out=ss[:], in_=sq[:],
                                            axis=mybir.AxisListType.X,
                                            op=mybir.AluOpType.add)
                    var = esmall.tile([P, nh], f32, tag=f"var_{sfx}",
                                      name=f"var_{sfx}")
                    nc.vector.tensor_tensor(out=var[:], in0=nmu[:],
                                            in1=nmu[:],
                                            op=mybir.AluOpType.mult)
                    nc.vector.tensor_tensor(out=var[:], in0=ss[:],
                                            in1=var[:],
                                            op=mybir.AluOpType.subtract)
                    sd = esmall.tile([P, nh], f32, tag=f"sd_{sfx}",
                                     name=f"sd_{sfx}")
                    nc.scalar.activation(
                        out=sd[:], in_=var[:],
                        func=mybir.ActivationFunctionType.Sqrt,
                        bias=eps_t2[:], scale=1.0)
                    rstd = esmall.tile([P, nh], f32, tag=f"rstd_{sfx}",
                                       name=f"rstd_{sfx}")
                    nc.vector.reciprocal(out=rstd[:], in_=sd[:])

                    # batched finals: y = relu((z + nmu) * rstd [*g + b])
                    t1 = ez.tile([P, nh, P], bf16, tag=f"t1_{sfx}",
                                 name=f"t1_{sfx}")
                    nc.vector.tensor_tensor(
                        out=t1[:], in0=z_ep[:],
                        in1=nmu[:, :, None].to_broadcast([P, nh, P]),
                        op=mybir.AluOpType.add)
                    y_all = ez.tile([P, nh, P], bf16, tag=f"y_{sfx}",
                                    name=f"y_{sfx}")
                    nc.vector.tensor_tensor(
                        out=y_all[:], in0=t1[:],
                        in1=rstd[:, :, None].to_broadcast([P, nh, P]),
                        op=mybir.AluOpType.mult)
                    if cfg["has_gk"]:
                        nc.vector.tensor_tensor(
                            out=y_all[:], in0=y_all[:],
                            in1=gk_b[:, None, :].to_broadcast([P, nh, P]),
                            op=mybir.AluOpType.mult)
                    if cfg["has_betak"]:
                        nc.vector.tensor_tensor(
                            out=y_all[:], in0=y_all[:],
                            in1=betak_b[:, None, :].to_broadcast([P, nh, P]),
                            op=mybir.AluOpType.add)
                    nc.scalar.activation(
                        out=y_all[:].rearrange("p s e -> p (s e)"),
                        in_=y_all[:].rearrange("p s e -> p (s e)"),
                        func=mybir.ActivationFunctionType.Relu)

                    # scatter-add this half's subtiles via one-hot matmuls
                    for s in range(h0, h0 + nh):
                        nc.tensor.matmul(out=acc_ps[:],
                                         lhsT=oh_sb[:, s, :],
                                         rhs=y_all[:, s - h0, :],
                                         start=(s == 0),
                                         stop=(s == K - 1))

                out_sb = ey.tile([P, OUT], f32, tag="out_sb")
                nc.vector.tensor_copy(out=out_sb[:], in_=acc_ps[:])
                nc.sync.dma_start(out=out_part[w * P:(w + 1) * P, :],
                                  in_=out_sb[:])

    nc.compile()
    return nc


# --------------------------------------------------------------------------
# entry point
# --------------------------------------------------------------------------
def _install_trace_hook():
    import sys, types, ctypes, contextlib
    if "antenv.axon_hooks" in sys.modules:
        return
    lib = ctypes.CDLL("/opt/axon/libaxon_pjrt.so")
    lib.axon_start_nrt_profile.argtypes = [ctypes.POINTER(ctypes.c_int64),
                                           ctypes.c_size_t]
    lib.axon_start_nrt_profile.restype = ctypes.c_int64
    lib.axon_stop_nrt_profile.argtypes = [ctypes.c_char_p]
    lib.axon_stop_nrt_profile.restype = ctypes.c_int64

    @contextlib.contextmanager
    def _hook(output_dir, device_ids):
        import jax
        jax.devices()
        if device_ids:
            ids = (ctypes.c_int64 * len(device_ids))(*device_ids)
            rc = lib.axon_start_nrt_profile(ids, len(device_ids))
        else:
            rc = lib.axon_start_nrt_profile(None, 0)
        if rc != 0:
            raise RuntimeError(f"axon_start_nrt_profile rc={rc}")
        try:
            yield
        finally:
            n = lib.axon_stop_nrt_profile(str(output_dir).encode())
            print(f"profile: {n} file(s) -> {output_dir}")

    mod = types.ModuleType("antenv.axon_hooks")
    mod.get_axon_ntff_profile_hook = lambda: _hook
    sys.modules["antenv.axon_hooks"] = mod
    from concourse import bass_utils
    bass_utils.upload_artifacts = lambda tmpdir: "local://skipped"


def kernel(**inputs):
    cfg, in_maps, pos = _prep(**inputs)
    key = (cfg["N"], cfg["GF"], cfg["OUT"], cfg["K"], cfg["has_bn"],
           cfg["has_gn"], cfg["has_betan"], cfg["has_bk"], cfg["has_gk"],
           cfg["has_betak"])
    if key not in _BUILD_CACHE:
        _BUILD_CACHE[key] = _build(cfg)
    nc = _BUILD_CACHE[key]

    if USE_SIM:
        from concourse import bass_interp
        sim = bass_interp.MultiCoreSim(nc, N_CORES)
        for c in range(N_CORES):
            for name, arr in in_maps[c].items():
                sim.cores[c].tensor(name)[:] = arr
        sim.simulate()
        parts = [np.array(sim.cores[c].tensor("out_part"))
                 for c in range(N_CORES)]
        exec_ns = None
    else:
        kw = {}
        if TRACE:
            _install_trace_hook()
            kw = dict(trace=True, tmpdir=TRACE_DIR)
        res = run_bass_kernel_spmd(nc, in_maps, list(range(N_CORES)), **kw)
        parts = [res.results[c]["out_part"] for c in range(N_CORES)]
        exec_ns = res.exec_time_ns
        kernel.last_exec_ns = exec_ns

    full = np.concatenate(parts, axis=0)
    out = full[pos[:cfg["N"]]]
    return out.astype(np.float32)


kernel.last_exec_ns = None

